# revision 2
# baseline (speedup 1.0000x reference)
"""GSN message-passing kernel for 8 Trainium2 NeuronCores (Bass/Tile).

Math (flat planes, TD = 2048*2048):
  B1 = [M0, M1, Z, H, H, H]
  Gf[p] = sigmoid(sum_j Wx[p%3,j]*B1[6*(p//3)+j] + bx[p%3])
  GM_n  = Gf[n*TD:(n+1)*TD] * B1[n*TD:(n+1)*TD]
  B2 = [GM0, H, GM1, H, GM2, H]
  Cf[p] = tanh(sum_j W[p%3,j]*B2[6*(p//3)+j] + b[p%3])
  out = H + M0 + M1 - G0*M0 - G1*M1 + G0*C0 + G1*C1 + G2*C2
applied for three node updates (M0,M1,H) = (f2,0,f0), (f0,0,f1), (f1,f0,f2).

Sharding: sequence(-flattened) dimension split into 8 overlapping shards
[A_c, A_c+L), A_c = 524289*c (multiple of 3 so the 6-element group phase is
identical on every core -> one SPMD NEFF).  Each core computes its full
dependency cone: gate streams at 2x ranges feeding candidate streams, plus
gates/candidates at the output range, all streamed tile-by-tile through SBUF.
Host pre-slices per-core inputs (plane-boundary crossings, v-forced regions
where sigmoid()==1 is required, and a few exact host-computed GM values are
baked into the input data, so a single uniform device program is exact).
"""
import sys
sys.path.insert(0, '/opt/trn_rl_repo')

import numpy as np

T = 2048
D = 2048
TD = T * D
N_CORES = 8
A_STEP = 524289          # shard start step (multiple of 3)
NT = 11                  # tiles per update
G2 = 128                 # stage-2 groups per partition
PP = 3 * G2              # out elems per partition per tile (384)
L = NT * 128 * PP        # 540672 per-core out length

ROW_SA = 12 * G2
ROW_SB = 12 * G2 + 18
ROW_MA = 6 * G2
ROW_MB = 6 * G2 + 9
ROW_C2 = 6 * G2 + 6
ROW_SG1 = 6 * G2 + 6
ROW_SG2 = 6 * G2 + 6
ROW_R1 = PP

LEN_SA = 4 * L
LEN_SB = 4 * L + 18
LEN_MA = 2 * L
LEN_MB = 2 * L + 9
LEN_C2 = 2 * L + 6
LEN_SG1 = 2 * L + 6
LEN_SG2 = 2 * L + 6
LEN_R1 = L

IN_SPECS = [("srcA", LEN_SA, ROW_SA, 12 * G2), ("srcB", LEN_SB, ROW_SB, 12 * G2),
            ("mA", LEN_MA, ROW_MA, 6 * G2), ("sG0", LEN_MA, ROW_MA, 6 * G2),
            ("mB", LEN_MB, ROW_MB, 6 * G2), ("c2s", LEN_C2, ROW_C2, 6 * G2),
            ("sG1", LEN_SG1, ROW_SG1, 6 * G2), ("sG2", LEN_SG2, ROW_SG2, 6 * G2),
            ("m0", LEN_R1, ROW_R1, ROW_R1), ("m1", LEN_R1, ROW_R1, ROW_R1),
            ("hh", LEN_R1, ROW_R1, ROW_R1)]

WX0, BX0, WW0, BB0, NWCOL = 0, 18, 21, 39, 42


# ---------------------------------------------------------------- host prep

def _take(plane_list, start, length, out):
    pos, off, n = 0, start, len(plane_list)
    while pos < length:
        pi = off // TD
        if pi >= n:
            out[pos:length] = 0.0
            return out
        lo = off - pi * TD
        take = min(TD - lo, length - pos)
        src = plane_list[pi]
        if src is None:
            out[pos:pos + take] = 0.0
        else:
            out[pos:pos + take] = src[lo:lo + take]
        pos += take
        off += take
    return out


def concat_take(plane_list, start, length):
    out = np.empty(length, np.float32)
    assert start >= 0
    _take(plane_list, start, length, out)
    return out


def solve_v(Wx, bx):
    """v in R^6 with sigmoid(Wx@v + bx) == 1.0f for all 3 rows."""
    target = np.full(3, 60.0, np.float64) - bx.astype(np.float64)
    v, *_ = np.linalg.lstsq(Wx.astype(np.float64), target, rcond=None)
    got = Wx.astype(np.float64) @ v + bx
    assert np.all(got > 30.0), got
    return v.astype(np.float32)


def _b1_at(y, M0, M1, H):
    y = np.asarray(y, np.int64)
    out = np.zeros(y.shape, np.float32)
    pi = y // TD
    lo = y - pi * TD
    for p, arr in enumerate([M0, M1, None, H, H, H]):
        m = pi == p
        if arr is not None and m.any():
            out[m] = arr[lo[m]]
    return out


def _gate_exact(pos, M0, M1, H, Wx, bx):
    pos = np.asarray(pos, np.int64)
    k = pos % 3
    i = pos // 3
    y = 6 * i[:, None] + np.arange(6)[None, :]
    B1v = _b1_at(y, M0, M1, H)
    z = (B1v * Wx[k]).sum(1) + bx[k]
    return (1.0 / (1.0 + np.exp(-z.astype(np.float64)))).astype(np.float32)


def prep_core_update(c, M0, M1, H, v, Wx, bx):
    A = A_STEP * c
    B1 = [M0, M1, None, H, H, H]

    e_cut = TD - 2 * A
    n_rowsA = 2 * L // 3
    r_cutA = max(0, min(n_rowsA, e_cut // 3))
    x_cut = 6 * r_cutA
    srcA = np.empty(LEN_SA, np.float32)
    if x_cut > 0:
        _take(B1, 4 * A, min(x_cut, LEN_SA), srcA)
    if x_cut < LEN_SA:
        nv = LEN_SA - x_cut
        srcA[x_cut:] = np.tile(v, nv // 6 + 1)[:nv]
    mA = np.empty(LEN_MA, np.float32)
    ec = max(0, min(e_cut, LEN_MA))
    if ec > 0:
        _take(B1, 2 * A, ec, mA)
    if ec < LEN_MA:
        mA[ec:] = concat_take([H], 2 * A + ec - TD, LEN_MA - ec)
    lo, hi = 3 * r_cutA, min(max(e_cut, 0), LEN_MA)
    if 0 <= lo < hi:   # exact GM0 in the straddle row
        ee = np.arange(lo, hi)
        mA[ee] = (_gate_exact(2 * A + ee, M0, M1, H, Wx, bx)
                  * _b1_at(2 * A + ee, M0, M1, H))
    lo2, hi2 = max(0, 2 * TD - 2 * A), min(2 * TD + 16 - 2 * A, LEN_MA)
    if lo2 < hi2:      # exact GM1 for the q=TD-1 taps past 2TD
        ee = np.arange(lo2, hi2)
        mA[ee] = (_gate_exact(2 * A + ee - TD, M0, M1, H, Wx, bx)
                  * _b1_at(2 * A + ee - TD, M0, M1, H))
    sG0 = concat_take(B1, 2 * A, LEN_MA)

    n_rowsB = LEN_MB // 3
    eB_cut = e_cut + 4
    rB_cut = max(0, min(n_rowsB, eB_cut // 3))
    rB_start = 2 if A == 0 else 0
    xB_cut = 6 * rB_cut
    srcB = np.empty(LEN_SB, np.float32)
    if xB_cut > 0:
        _take(B1, 2 * TD + 4 * A - 8, min(xB_cut, LEN_SB), srcB)
    if xB_cut < LEN_SB:
        nv = LEN_SB - xB_cut
        srcB[xB_cut:] = np.tile(v, nv // 6 + 1)[:nv]
    if rB_start > 0:
        srcB[:6 * rB_start] = np.tile(v, rB_start)
    mB = np.empty(LEN_MB, np.float32)
    st = 2 * A - 4
    if st >= 0:
        _take([M1, H], st, LEN_MB, mB)
    else:
        mB[:4] = H[TD - 4:]
        _take([M1, H], 0, LEN_MB - 4, mB[4:])
    lo, hi = 3 * rB_cut, min(max(eB_cut, 0), LEN_MB)
    if 0 <= lo < hi:   # exact GM1 in the straddle row
        ee = np.arange(lo, hi)
        gp = TD + 2 * A - 4 + ee
        mB[ee] = (_gate_exact(gp, M0, M1, H, Wx, bx) * _b1_at(gp, M0, M1, H))
    if rB_start > 0:   # c==0: e' in {4,5} of forced row 1 need exact GM1
        ee = np.arange(4, 6)
        gp = TD + 2 * A - 4 + ee
        mB[ee] = (_gate_exact(gp, M0, M1, H, Wx, bx) * _b1_at(gp, M0, M1, H))

    c2s = (concat_take([None, H], 2 * A - 4, LEN_C2) if 2 * A >= 4
           else concat_take([H, None, H], TD + 2 * A - 4, LEN_C2))
    sG1 = (concat_take([None, H, H], 2 * A - 2, LEN_SG1) if 2 * A >= 2
           else concat_take([M1, None, H, H], TD + 2 * A - 2, LEN_SG1))
    sG2 = (concat_take([H, H], 2 * A - 4, LEN_SG2) if 2 * A >= 4
           else concat_take([H, H, H], TD + 2 * A - 4, LEN_SG2))

    m0 = concat_take([M0], A, LEN_R1)
    m1 = concat_take([M1], A, LEN_R1)
    hh = concat_take([H], A, LEN_R1)
    return dict(srcA=srcA, srcB=srcB, mA=mA, sG0=sG0, mB=mB, c2s=c2s,
                sG1=sG1, sG2=sG2, m0=m0, m1=m1, hh=hh)


def build_weight_array(Wx, bx, W, b):
    w = np.zeros(NWCOL, np.float32)
    w[WX0:WX0 + 18] = np.asarray(Wx, np.float32).reshape(-1)
    w[BX0:BX0 + 3] = bx
    w[WW0:WW0 + 18] = np.asarray(W, np.float32).reshape(-1)
    w[BB0:BB0 + 3] = b
    return np.tile(w[None, :], (128, 1))


def make_in_maps(feature, W_w, W_b, Wx_w, Wx_b):
    f = [np.ascontiguousarray(np.asarray(feature, np.float32)[i].reshape(-1))
         for i in range(3)]
    Zp = np.zeros(TD, np.float32)
    Wx = np.asarray(Wx_w, np.float32)
    bx = np.asarray(Wx_b, np.float32)
    v = solve_v(Wx, bx)
    wt = build_weight_array(Wx, bx, W_w, W_b)
    updates = [(f[2], Zp, f[0]), (f[0], Zp, f[1]), (f[1], f[0], f[2])]
    in_maps = []
    for c in range(N_CORES):
        m = {"wt": wt}
        for u, (M0, M1, H) in enumerate(updates):
            inp = prep_core_update(c, M0, M1, H, v, Wx, bx)
            for nm, arr in inp.items():
                m[f"{nm}{u}"] = arr
        in_maps.append(m)
    return in_maps


# ---------------------------------------------------------------- device code

def build_nc():
    import concourse.bass as bass
    import concourse.bacc as bacc_mod
    import concourse.mybir as mybir
    from concourse.tile import TileContext

    F32 = mybir.dt.float32
    MULT = mybir.AluOpType.mult
    ADD = mybir.AluOpType.add
    SIGMOID = mybir.ActivationFunctionType.Sigmoid
    TANH = mybir.ActivationFunctionType.Tanh

    nc = bacc_mod.Bacc()
    wt_d = nc.dram_tensor("wt", [128, NWCOL], F32, kind="ExternalInput")
    ins = {}
    for u in range(3):
        for nm, ln, row, stride in IN_SPECS:
            ins[(u, nm)] = nc.dram_tensor(f"{nm}{u}", [ln], F32, kind="ExternalInput")
    out_d = nc.dram_tensor("out", [3, L], F32, kind="ExternalOutput")

    def dram_ap(handle, off, row, stride):
        return bass.AP(handle, off, [[stride, 128], [1, row]])

    def grouped(wtile, out_tile, in_ap_of, ngroups, w_col, b_col):
        """out[:, 3g+k] = sum_j w[k,j]*in(6g+j) + b[k]; in_ap_of(j) gives the
        strided tap AP for offset j."""
        for k in range(3):
            o = out_tile[:, k:3 * (ngroups - 1) + k + 1:3]
            nc.vector.tensor_scalar(o, in_ap_of(0),
                                    wtile[:, w_col + 6 * k:w_col + 6 * k + 1],
                                    wtile[:, b_col + k:b_col + k + 1], MULT, ADD)
            for j in range(1, 6):
                nc.vector.scalar_tensor_tensor(
                    o, in_ap_of(j),
                    wtile[:, w_col + 6 * k + j:w_col + 6 * k + j + 1], o, MULT, ADD)

    with TileContext(nc) as tc:
        with tc.tile_pool(name="wp", bufs=1) as wpool, \
             tc.tile_pool(name="io", bufs=2) as pool:
            wtile = wpool.tile([128, NWCOL], F32)
            nc.sync.dma_start(wtile[:, :], wt_d[:, :])
            for u in range(3):
                for t in range(NT):
                    tin = {}
                    for nm, ln, row, stride in IN_SPECS:
                        tl = pool.tile([128, row], F32, tag=nm)
                        nc.sync.dma_start(
                            tl[:, :], dram_ap(ins[(u, nm)], t * 128 * stride, row, stride))
                        tin[nm] = tl

                    def taps(tile, base, ng):
                        return lambda j: tile[:, base + j:base + j + 6 * (ng - 1) + 1:6]

                    gApre = pool.tile([128, 6 * G2], F32, tag="gApre")
                    grouped(wtile, gApre, taps(tin["srcA"], 0, 2 * G2), 2 * G2, WX0, BX0)
                    gA = pool.tile([128, 6 * G2], F32, tag="gA")
                    nc.scalar.activation(gA[:, :], gApre[:, :], SIGMOID)
                    c0s = pool.tile([128, 6 * G2], F32, tag="c0s")
                    nc.vector.tensor_mul(c0s[:, :], gA[:, :], tin["mA"][:, :])

                    gBpre = pool.tile([128, 6 * G2 + 9], F32, tag="gBpre")
                    grouped(wtile, gBpre, taps(tin["srcB"], 0, 2 * G2 + 3), 2 * G2 + 3, WX0, BX0)
                    gB = pool.tile([128, 6 * G2 + 9], F32, tag="gB")
                    nc.scalar.activation(gB[:, :], gBpre[:, :], SIGMOID)
                    c1s = pool.tile([128, 6 * G2 + 9], F32, tag="c1s")
                    nc.vector.tensor_mul(c1s[:, :], gB[:, :], tin["mB"][:, :])

                    g0pre = pool.tile([128, PP], F32, tag="g0pre")
                    grouped(wtile, g0pre, taps(tin["sG0"], 0, G2), G2, WX0, BX0)
                    g0 = pool.tile([128, PP], F32, tag="g0")
                    nc.scalar.activation(g0[:, :], g0pre[:, :], SIGMOID)
                    g1pre = pool.tile([128, PP + 3], F32, tag="g1pre")
                    grouped(wtile, g1pre, taps(tin["sG1"], 0, G2 + 1), G2 + 1, WX0, BX0)
                    g1 = pool.tile([128, PP + 3], F32, tag="g1")
                    nc.scalar.activation(g1[:, :], g1pre[:, :], SIGMOID)
                    g2pre = pool.tile([128, PP + 3], F32, tag="g2pre")
                    grouped(wtile, g2pre, taps(tin["sG2"], 0, G2 + 1), G2 + 1, WX0, BX0)
                    g2 = pool.tile([128, PP + 3], F32, tag="g2")
                    nc.scalar.activation(g2[:, :], g2pre[:, :], SIGMOID)

                    c0pre = pool.tile([128, PP], F32, tag="c0pre")
                    grouped(wtile, c0pre, taps(c0s, 0, G2), G2, WW0, BB0)
                    c0 = pool.tile([128, PP], F32, tag="c0")
                    nc.scalar.activation(c0[:, :], c0pre[:, :], TANH)
                    c1pre = pool.tile([128, PP + 3], F32, tag="c1pre")
                    grouped(wtile, c1pre, taps(c1s, 2, G2 + 1), G2 + 1, WW0, BB0)
                    c1 = pool.tile([128, PP + 3], F32, tag="c1")
                    nc.scalar.activation(c1[:, :], c1pre[:, :], TANH)
                    c2pre = pool.tile([128, PP + 3], F32, tag="c2pre")
                    grouped(wtile, c2pre, taps(tin["c2s"], 0, G2 + 1), G2 + 1, WW0, BB0)
                    c2 = pool.tile([128, PP + 3], F32, tag="c2")
                    nc.scalar.activation(c2[:, :], c2pre[:, :], TANH)

                    tmp0 = pool.tile([128, PP], F32, tag="tmp0")
                    nc.vector.tensor_sub(tmp0[:, :], c0[:, :], tin["m0"][:, :])
                    nc.vector.tensor_mul(tmp0[:, :], g0[:, :], tmp0[:, :])
                    tmp1 = pool.tile([128, PP], F32, tag="tmp1")
                    nc.vector.tensor_sub(tmp1[:, :], c1[:, 1:PP + 1], tin["m1"][:, :])
                    nc.vector.tensor_mul(tmp1[:, :], g1[:, 1:PP + 1], tmp1[:, :])
                    tmp2 = pool.tile([128, PP], F32, tag="tmp2")
                    nc.vector.tensor_mul(tmp2[:, :], g2[:, 2:PP + 2], c2[:, 2:PP + 2])
                    ot = pool.tile([128, PP], F32, tag="ot")
                    nc.vector.tensor_add(ot[:, :], tin["hh"][:, :], tin["m0"][:, :])
                    nc.vector.tensor_add(ot[:, :], ot[:, :], tin["m1"][:, :])
                    nc.vector.tensor_add(ot[:, :], ot[:, :], tmp0[:, :])
                    nc.vector.tensor_add(ot[:, :], ot[:, :], tmp1[:, :])
                    nc.vector.tensor_add(ot[:, :], ot[:, :], tmp2[:, :])
                    nc.sync.dma_start(dram_ap(out_d, u * L + t * 128 * PP, PP, PP),
                                      ot[:, :])
    nc.finalize()
    return nc


# ---------------------------------------------------------------- runner

_STATE = {}


def _get_exec():
    """Build nc once and return (sharded_fn, in_names, mesh, sharding)."""
    if "fn" in _STATE:
        return _STATE
    import jax
    import numpy as _np
    import concourse.mybir as mybir
    from jax.sharding import Mesh, PartitionSpec, NamedSharding
    from jax.experimental.shard_map import shard_map
    from concourse.bass2jax import install_neuronx_cc_hook, _bass_exec_p, \
        partition_id_tensor

    nc = build_nc()
    install_neuronx_cc_hook()

    pname = nc.partition_id_tensor.name if nc.partition_id_tensor else None
    in_names, out_names, out_avals = [], [], []
    for alloc in nc.m.functions[0].allocations:
        if not isinstance(alloc, mybir.MemoryLocationSet):
            continue
        name = alloc.memorylocations[0].name
        if alloc.kind == "ExternalInput":
            if name != pname:
                in_names.append(name)
        elif alloc.kind == "ExternalOutput":
            out_names.append(name)
            out_avals.append(jax.core.ShapedArray(tuple(alloc.tensor_shape),
                                                  mybir.dt.np(alloc.dtype)))
    n_params = len(in_names)
    all_names = tuple(in_names) + tuple(out_names) + ((pname,) if pname else ())

    def _body(*args):
        ops = list(args)
        if pname:
            ops.append(partition_id_tensor())
        outs = _bass_exec_p.bind(
            *ops, out_avals=tuple(out_avals), in_names=all_names,
            out_names=tuple(out_names), lowering_input_output_aliases=(),
            sim_require_finite=False, sim_require_nnan=False, nc=nc)
        return tuple(outs)

    devices = jax.devices()[:N_CORES]
    mesh = Mesh(np.asarray(devices), ("core",))
    nin = n_params + len(out_names)
    fn = jax.jit(shard_map(_body, mesh=mesh,
                           in_specs=(PartitionSpec("core"),) * nin,
                           out_specs=(PartitionSpec("core"),) * len(out_names),
                           check_rep=False),
                 keep_unused=True)
    _STATE.update(fn=fn, in_names=in_names, out_names=out_names,
                  out_avals=out_avals, mesh=mesh,
                  sharding=NamedSharding(mesh, PartitionSpec("core")))
    return _STATE


def _run(in_maps):
    st = _get_exec()
    concat_in = [np.concatenate([np.asarray(in_maps[c][n]).reshape(-1)
                                 if np.asarray(in_maps[c][n]).ndim == 1
                                 else np.asarray(in_maps[c][n])
                                 for c in range(N_CORES)], axis=0)
                 for n in st["in_names"]]
    zeros = [np.zeros((N_CORES * a.shape[0], *a.shape[1:]), a.dtype)
             for a in st["out_avals"]]
    outs = st["fn"](*concat_in, *zeros)
    res = []
    for c in range(N_CORES):
        d = {}
        for i, n in enumerate(st["out_names"]):
            a = st["out_avals"][i]
            d[n] = np.asarray(outs[i]).reshape(N_CORES, *a.shape)[c]
        res.append(d)
    return res


def assemble(results):
    outs = np.zeros((3, TD), np.float32)
    for c in range(N_CORES):
        A = A_STEP * c
        n = min(L, TD - A)
        o = results[c]["out"]
        for u in range(3):
            outs[u, A:A + n] = o[u][:n]
    return outs.reshape(3, T, D)


def kernel(feature, W_w, W_b, Wx_w, Wx_b):
    in_maps = make_in_maps(feature, W_w, W_b, Wx_w, Wx_b)
    res = _run(in_maps)
    return assemble(res)


def hw_exec_time_ns(in_maps=None, iters=24):
    """Amortized per-execution device time: inputs resident on device,
    `iters` executions dispatched back-to-back (pipelined), minus the
    single-dispatch overhead baseline."""
    import time, jax
    st = _get_exec()
    if in_maps is None:
        rng = np.random.default_rng(0)
        feature = rng.standard_normal((3, T, D), dtype=np.float32)
        s = 1.0 / np.sqrt(6)
        in_maps = make_in_maps(feature,
                               rng.uniform(-s, s, (3, 6)).astype(np.float32),
                               rng.uniform(-s, s, 3).astype(np.float32),
                               rng.uniform(-s, s, (3, 6)).astype(np.float32),
                               rng.uniform(-s, s, 3).astype(np.float32))
    concat_in = [np.concatenate([np.asarray(in_maps[c][n]) for c in range(N_CORES)],
                                axis=0) for n in st["in_names"]]
    zeros = [np.zeros((N_CORES * a.shape[0], *a.shape[1:]), a.dtype)
             for a in st["out_avals"]]
    xd = [jax.device_put(a, st["sharding"]) for a in concat_in]
    zd = [jax.device_put(a, st["sharding"]) for a in zeros]
    for a in xd:
        a.block_until_ready()
    o = st["fn"](*xd, *zd)
    o[0].block_until_ready()
    t0 = time.time()
    o = st["fn"](*xd, *zd)
    o[0].block_until_ready()
    t1 = time.time()
    single = t1 - t0
    t0 = time.time()
    outs = [st["fn"](*xd, *zd) for _ in range(iters)]
    outs[-1][0].block_until_ready()
    t1 = time.time()
    per = (t1 - t0 - single) / (iters - 1)
    return max(per, 0.0) * 1e9, single * 1e9


# revision 5
# speedup vs baseline: 1.3471x; 1.3471x over previous
"""GSN message-passing kernel for 8 Trainium2 NeuronCores (Bass/Tile).

Math (flat planes, TD = 2048*2048):
  B1 = [M0, M1, Z, H, H, H]
  Gf[p] = sigmoid(sum_j Wx[p%3,j]*B1[6*(p//3)+j] + bx[p%3])
  GM_n  = Gf[n*TD:(n+1)*TD] * B1[n*TD:(n+1)*TD]
  B2 = [GM0, H, GM1, H, GM2, H]
  Cf[p] = tanh(sum_j W[p%3,j]*B2[6*(p//3)+j] + b[p%3])
  out = H + M0 + M1 - G0*M0 - G1*M1 + G0*C0 + G1*C1 + G2*C2
applied for three node updates (M0,M1,H) = (f2,0,f0), (f0,0,f1), (f1,f0,f2).

Sharding: sequence(-flattened) dimension split into 8 overlapping shards
[A_c, A_c+L), A_c = 524289*c (multiple of 3 so the 6-element group phase is
identical on every core -> one SPMD NEFF).  Each core computes its full
dependency cone: gate streams at 2x ranges feeding candidate streams, plus
gates/candidates at the output range, all streamed tile-by-tile through SBUF.
Host pre-slices per-core inputs (plane-boundary crossings, v-forced regions
where sigmoid()==1 is required, and a few exact host-computed GM values are
baked into the input data, so a single uniform device program is exact).
"""
import sys
sys.path.insert(0, '/opt/trn_rl_repo')

import numpy as np

T = 2048
D = 2048
TD = T * D
N_CORES = 8
A_STEP = 524289          # shard start step (multiple of 3)
NT = 11                  # tiles per update
G2 = 128                 # stage-2 groups per partition
PP = 3 * G2              # out elems per partition per tile (384)
L = NT * 128 * PP        # 540672 per-core out length

ROW_SA = 12 * G2
ROW_SB = 12 * G2 + 18
ROW_MA = 6 * G2
ROW_MB = 6 * G2 + 9
ROW_C2 = 6 * G2 + 6
ROW_SG1 = 6 * G2 + 6
ROW_SG2 = 6 * G2 + 6
ROW_R1 = PP

LEN_SA = 4 * L
LEN_SB = 4 * L + 18
LEN_MA = 2 * L
LEN_MB = 2 * L + 9
LEN_C2 = 2 * L + 6
LEN_SG1 = 2 * L + 6
LEN_SG2 = 2 * L + 6
LEN_R1 = L

IN_SPECS = [("srcA", LEN_SA, ROW_SA, 12 * G2), ("srcB", LEN_SB, ROW_SB, 12 * G2),
            ("mA", LEN_MA, ROW_MA, 6 * G2), ("sG0", LEN_MA, ROW_MA, 6 * G2),
            ("mB", LEN_MB, ROW_MB, 6 * G2), ("c2s", LEN_C2, ROW_C2, 6 * G2),
            ("sG1", LEN_SG1, ROW_SG1, 6 * G2), ("sG2", LEN_SG2, ROW_SG2, 6 * G2),
            ("m0", LEN_R1, ROW_R1, ROW_R1), ("m1", LEN_R1, ROW_R1, ROW_R1),
            ("hh", LEN_R1, ROW_R1, ROW_R1)]

WX0, BX0, WW0, BB0, NWCOL = 0, 18, 21, 39, 42


# ---------------------------------------------------------------- host prep

DT = None  # stream dtype, set in make_in_maps (ml_dtypes.bfloat16)


def _take(plane_list, start, length, out):
    pos, off, n = 0, start, len(plane_list)
    while pos < length:
        pi = off // TD
        if pi >= n:
            out[pos:length] = 0.0
            return out
        lo = off - pi * TD
        take = min(TD - lo, length - pos)
        src = plane_list[pi]
        if src is None:
            out[pos:pos + take] = 0.0
        else:
            out[pos:pos + take] = src[lo:lo + take]
        pos += take
        off += take
    return out


def concat_take(plane_list, start, length):
    out = np.empty(length, DT)
    assert start >= 0
    _take(plane_list, start, length, out)
    return out


def solve_v(Wx, bx):
    """v in R^6 with sigmoid(Wx@v + bx) == 1.0f for all 3 rows."""
    target = np.full(3, 60.0, np.float64) - bx.astype(np.float64)
    v, *_ = np.linalg.lstsq(Wx.astype(np.float64), target, rcond=None)
    got = Wx.astype(np.float64) @ v + bx
    assert np.all(got > 30.0), got
    return v.astype(np.float32)


def _b1_at(y, M0, M1, H):
    y = np.asarray(y, np.int64)
    out = np.zeros(y.shape, np.float32)  # f32 math on bf16-rounded values
    pi = y // TD
    lo = y - pi * TD
    for p, arr in enumerate([M0, M1, None, H, H, H]):
        m = pi == p
        if arr is not None and m.any():
            out[m] = arr[lo[m]]
    return out


def _gate_exact(pos, M0, M1, H, Wx, bx):
    pos = np.asarray(pos, np.int64)
    k = pos % 3
    i = pos // 3
    y = 6 * i[:, None] + np.arange(6)[None, :]
    B1v = _b1_at(y, M0, M1, H)
    z = (B1v * Wx[k]).sum(1) + bx[k]
    return (1.0 / (1.0 + np.exp(-z.astype(np.float64)))).astype(np.float32)


def prep_core_update(c, M0, M1, H, v, Wx, bx):
    A = A_STEP * c
    B1 = [M0, M1, None, H, H, H]

    e_cut = TD - 2 * A
    n_rowsA = 2 * L // 3
    r_cutA = max(0, min(n_rowsA, e_cut // 3))
    x_cut = 6 * r_cutA
    srcA = np.empty(LEN_SA, DT)
    if x_cut > 0:
        _take(B1, 4 * A, min(x_cut, LEN_SA), srcA)
    if x_cut < LEN_SA:
        nv = LEN_SA - x_cut
        srcA[x_cut:] = np.tile(v, nv // 6 + 1)[:nv]
    mA = np.empty(LEN_MA, DT)
    ec = max(0, min(e_cut, LEN_MA))
    if ec > 0:
        _take(B1, 2 * A, ec, mA)
    if ec < LEN_MA:
        mA[ec:] = concat_take([H], 2 * A + ec - TD, LEN_MA - ec)
    lo, hi = 3 * r_cutA, min(max(e_cut, 0), LEN_MA)
    if 0 <= lo < hi:   # exact GM0 in the straddle row
        ee = np.arange(lo, hi)
        mA[ee] = (_gate_exact(2 * A + ee, M0, M1, H, Wx, bx)
                  * _b1_at(2 * A + ee, M0, M1, H))
    lo2, hi2 = max(0, 2 * TD - 2 * A), min(2 * TD + 16 - 2 * A, LEN_MA)
    if lo2 < hi2:      # exact GM1 for the q=TD-1 taps past 2TD
        ee = np.arange(lo2, hi2)
        mA[ee] = (_gate_exact(2 * A + ee - TD, M0, M1, H, Wx, bx)
                  * _b1_at(2 * A + ee - TD, M0, M1, H))
    sG0 = concat_take(B1, 2 * A, LEN_MA)

    n_rowsB = LEN_MB // 3
    eB_cut = e_cut + 4
    rB_cut = max(0, min(n_rowsB, eB_cut // 3))
    rB_start = 2 if A == 0 else 0
    xB_cut = 6 * rB_cut
    srcB = np.empty(LEN_SB, DT)
    if xB_cut > 0:
        _take(B1, 2 * TD + 4 * A - 8, min(xB_cut, LEN_SB), srcB)
    if xB_cut < LEN_SB:
        nv = LEN_SB - xB_cut
        srcB[xB_cut:] = np.tile(v, nv // 6 + 1)[:nv]
    if rB_start > 0:
        srcB[:6 * rB_start] = np.tile(v, rB_start)
    mB = np.empty(LEN_MB, DT)
    st = 2 * A - 4
    if st >= 0:
        _take([M1, H], st, LEN_MB, mB)
    else:
        mB[:4] = H[TD - 4:]
        _take([M1, H], 0, LEN_MB - 4, mB[4:])
    lo, hi = 3 * rB_cut, min(max(eB_cut, 0), LEN_MB)
    if 0 <= lo < hi:   # exact GM1 in the straddle row
        ee = np.arange(lo, hi)
        gp = TD + 2 * A - 4 + ee
        mB[ee] = (_gate_exact(gp, M0, M1, H, Wx, bx) * _b1_at(gp, M0, M1, H))
    if rB_start > 0:   # c==0: e' in {4,5} of forced row 1 need exact GM1
        ee = np.arange(4, 6)
        gp = TD + 2 * A - 4 + ee
        mB[ee] = (_gate_exact(gp, M0, M1, H, Wx, bx) * _b1_at(gp, M0, M1, H))

    c2s = (concat_take([None, H], 2 * A - 4, LEN_C2) if 2 * A >= 4
           else concat_take([H, None, H], TD + 2 * A - 4, LEN_C2))
    sG1 = (concat_take([None, H, H], 2 * A - 2, LEN_SG1) if 2 * A >= 2
           else concat_take([M1, None, H, H], TD + 2 * A - 2, LEN_SG1))
    sG2 = (concat_take([H, H], 2 * A - 4, LEN_SG2) if 2 * A >= 4
           else concat_take([H, H, H], TD + 2 * A - 4, LEN_SG2))

    m0 = concat_take([M0], A, LEN_R1)
    m1 = concat_take([M1], A, LEN_R1)
    hh = concat_take([H], A, LEN_R1)
    return dict(srcA=srcA, srcB=srcB, mA=mA, sG0=sG0, mB=mB, c2s=c2s,
                sG1=sG1, sG2=sG2, m0=m0, m1=m1, hh=hh)


def build_weight_array(Wx, bx, W, b):
    w = np.zeros(NWCOL, np.float32)
    w[WX0:WX0 + 18] = np.asarray(Wx, np.float32).reshape(-1)
    w[BX0:BX0 + 3] = bx
    w[WW0:WW0 + 18] = np.asarray(W, np.float32).reshape(-1)
    w[BB0:BB0 + 3] = b
    return np.tile(w[None, :], (128, 1))


def make_in_maps(feature, W_w, W_b, Wx_w, Wx_b):
    global DT
    import ml_dtypes
    DT = np.dtype(ml_dtypes.bfloat16)
    f = [np.ascontiguousarray(np.asarray(feature, np.float32)[i].reshape(-1)).astype(DT)
         for i in range(3)]
    Zp = np.zeros(TD, DT)
    Wx = np.asarray(Wx_w, np.float32)
    bx = np.asarray(Wx_b, np.float32)
    v = solve_v(Wx, bx).astype(DT)
    wt = build_weight_array(Wx, bx, W_w, W_b)
    updates = [(f[2], Zp, f[0]), (f[0], Zp, f[1]), (f[1], f[0], f[2])]
    in_maps = []
    for c in range(N_CORES):
        m = {"wt": wt}
        for u, (M0, M1, H) in enumerate(updates):
            inp = prep_core_update(c, M0, M1, H, v, Wx, bx)
            for nm, arr in inp.items():
                m[f"{nm}{u}"] = arr
        in_maps.append(m)
    return in_maps


# ---------------------------------------------------------------- device code

def build_nc():
    import concourse.bass as bass
    import concourse.bacc as bacc_mod
    import concourse.mybir as mybir
    from concourse.tile import TileContext

    F32 = mybir.dt.float32
    BF16 = mybir.dt.bfloat16
    MULT = mybir.AluOpType.mult
    ADD = mybir.AluOpType.add
    SIGMOID = mybir.ActivationFunctionType.Sigmoid
    TANH = mybir.ActivationFunctionType.Tanh

    nc = bacc_mod.Bacc()
    wt_d = nc.dram_tensor("wt", [128, NWCOL], F32, kind="ExternalInput")
    ins = {}
    for u in range(3):
        for nm, ln, row, stride in IN_SPECS:
            ins[(u, nm)] = nc.dram_tensor(f"{nm}{u}", [ln], BF16, kind="ExternalInput")
    out_d = nc.dram_tensor("out", [3, L], F32, kind="ExternalOutput")

    def dram_ap(handle, off, row, stride):
        return bass.AP(handle, off, [[stride, 128], [1, row]])

    def grouped(wtile, out_tile, in_ap_of, ngroups, w_col, b_col):
        """out[:, 3g+k] = sum_j w[k,j]*in(6g+j) + b[k]; in_ap_of(j) gives the
        strided tap AP for offset j."""
        for k in range(3):
            o = out_tile[:, k:3 * (ngroups - 1) + k + 1:3]
            nc.vector.tensor_scalar(o, in_ap_of(0),
                                    wtile[:, w_col + 6 * k:w_col + 6 * k + 1],
                                    wtile[:, b_col + k:b_col + k + 1], MULT, ADD)
            for j in range(1, 6):
                nc.vector.scalar_tensor_tensor(
                    o, in_ap_of(j),
                    wtile[:, w_col + 6 * k + j:w_col + 6 * k + j + 1], o, MULT, ADD)

    with TileContext(nc) as tc:
        with tc.tile_pool(name="wp", bufs=1) as wpool, \
             tc.tile_pool(name="io", bufs=2) as pool:
            wtile = wpool.tile([128, NWCOL], F32)
            nc.sync.dma_start(wtile[:, :], wt_d[:, :])
            for u in range(3):
                for t in range(NT):
                    tin = {}
                    for nm, ln, row, stride in IN_SPECS:
                        tl = pool.tile([128, row], BF16, tag=nm)
                        nc.sync.dma_start(
                            tl[:, :], dram_ap(ins[(u, nm)], t * 128 * stride, row, stride))
                        tin[nm] = tl

                    def taps(tile, base, ng):
                        return lambda j: tile[:, base + j:base + j + 6 * (ng - 1) + 1:6]

                    gApre = pool.tile([128, 6 * G2], F32, tag="gApre")
                    grouped(wtile, gApre, taps(tin["srcA"], 0, 2 * G2), 2 * G2, WX0, BX0)
                    gA = pool.tile([128, 6 * G2], F32, tag="gA")
                    nc.scalar.activation(gA[:, :], gApre[:, :], SIGMOID)
                    c0s = pool.tile([128, 6 * G2], F32, tag="c0s")
                    nc.gpsimd.tensor_mul(c0s[:, :], gA[:, :], tin["mA"][:, :])

                    gBpre = pool.tile([128, 6 * G2 + 9], F32, tag="gBpre")
                    grouped(wtile, gBpre, taps(tin["srcB"], 0, 2 * G2 + 3), 2 * G2 + 3, WX0, BX0)
                    gB = pool.tile([128, 6 * G2 + 9], F32, tag="gB")
                    nc.scalar.activation(gB[:, :], gBpre[:, :], SIGMOID)
                    c1s = pool.tile([128, 6 * G2 + 9], F32, tag="c1s")
                    nc.gpsimd.tensor_mul(c1s[:, :], gB[:, :], tin["mB"][:, :])

                    g0pre = pool.tile([128, PP], F32, tag="g0pre")
                    grouped(wtile, g0pre, taps(tin["sG0"], 0, G2), G2, WX0, BX0)
                    g0 = pool.tile([128, PP], F32, tag="g0")
                    nc.scalar.activation(g0[:, :], g0pre[:, :], SIGMOID)
                    g1pre = pool.tile([128, PP + 3], F32, tag="g1pre")
                    grouped(wtile, g1pre, taps(tin["sG1"], 0, G2 + 1), G2 + 1, WX0, BX0)
                    g1 = pool.tile([128, PP + 3], F32, tag="g1")
                    nc.scalar.activation(g1[:, :], g1pre[:, :], SIGMOID)
                    g2pre = pool.tile([128, PP + 3], F32, tag="g2pre")
                    grouped(wtile, g2pre, taps(tin["sG2"], 0, G2 + 1), G2 + 1, WX0, BX0)
                    g2 = pool.tile([128, PP + 3], F32, tag="g2")
                    nc.scalar.activation(g2[:, :], g2pre[:, :], SIGMOID)

                    c0pre = pool.tile([128, PP], F32, tag="c0pre")
                    grouped(wtile, c0pre, taps(c0s, 0, G2), G2, WW0, BB0)
                    c0 = pool.tile([128, PP], F32, tag="c0")
                    nc.scalar.activation(c0[:, :], c0pre[:, :], TANH)
                    c1pre = pool.tile([128, PP + 3], F32, tag="c1pre")
                    grouped(wtile, c1pre, taps(c1s, 2, G2 + 1), G2 + 1, WW0, BB0)
                    c1 = pool.tile([128, PP + 3], F32, tag="c1")
                    nc.scalar.activation(c1[:, :], c1pre[:, :], TANH)
                    c2pre = pool.tile([128, PP + 3], F32, tag="c2pre")
                    grouped(wtile, c2pre, taps(tin["c2s"], 0, G2 + 1), G2 + 1, WW0, BB0)
                    c2 = pool.tile([128, PP + 3], F32, tag="c2")
                    nc.scalar.activation(c2[:, :], c2pre[:, :], TANH)

                    tmp0 = pool.tile([128, PP], F32, tag="tmp0")
                    nc.gpsimd.tensor_sub(tmp0[:, :], c0[:, :], tin["m0"][:, :])
                    nc.gpsimd.tensor_mul(tmp0[:, :], g0[:, :], tmp0[:, :])
                    tmp1 = pool.tile([128, PP], F32, tag="tmp1")
                    nc.gpsimd.tensor_sub(tmp1[:, :], c1[:, 1:PP + 1], tin["m1"][:, :])
                    nc.gpsimd.tensor_mul(tmp1[:, :], g1[:, 1:PP + 1], tmp1[:, :])
                    tmp2 = pool.tile([128, PP], F32, tag="tmp2")
                    nc.gpsimd.tensor_mul(tmp2[:, :], g2[:, 2:PP + 2], c2[:, 2:PP + 2])
                    ot = pool.tile([128, PP], F32, tag="ot")
                    nc.gpsimd.tensor_add(ot[:, :], tin["hh"][:, :], tin["m0"][:, :])
                    nc.gpsimd.tensor_add(ot[:, :], ot[:, :], tin["m1"][:, :])
                    nc.gpsimd.tensor_add(ot[:, :], ot[:, :], tmp0[:, :])
                    nc.gpsimd.tensor_add(ot[:, :], ot[:, :], tmp1[:, :])
                    nc.gpsimd.tensor_add(ot[:, :], ot[:, :], tmp2[:, :])
                    nc.sync.dma_start(dram_ap(out_d, u * L + t * 128 * PP, PP, PP),
                                      ot[:, :])
    nc.finalize()
    return nc


# ---------------------------------------------------------------- runner

_STATE = {}


def _get_exec():
    """Build nc once and return (sharded_fn, in_names, mesh, sharding)."""
    if "fn" in _STATE:
        return _STATE
    import jax
    import numpy as _np
    import concourse.mybir as mybir
    from jax.sharding import Mesh, PartitionSpec, NamedSharding
    from jax.experimental.shard_map import shard_map
    from concourse.bass2jax import install_neuronx_cc_hook, _bass_exec_p, \
        partition_id_tensor

    nc = build_nc()
    install_neuronx_cc_hook()

    pname = nc.partition_id_tensor.name if nc.partition_id_tensor else None
    in_names, out_names, out_avals = [], [], []
    for alloc in nc.m.functions[0].allocations:
        if not isinstance(alloc, mybir.MemoryLocationSet):
            continue
        name = alloc.memorylocations[0].name
        if alloc.kind == "ExternalInput":
            if name != pname:
                in_names.append(name)
        elif alloc.kind == "ExternalOutput":
            out_names.append(name)
            out_avals.append(jax.core.ShapedArray(tuple(alloc.tensor_shape),
                                                  mybir.dt.np(alloc.dtype)))
    n_params = len(in_names)
    all_names = tuple(in_names) + tuple(out_names) + ((pname,) if pname else ())

    def _body(*args):
        ops = list(args)
        if pname:
            ops.append(partition_id_tensor())
        outs = _bass_exec_p.bind(
            *ops, out_avals=tuple(out_avals), in_names=all_names,
            out_names=tuple(out_names), lowering_input_output_aliases=(),
            sim_require_finite=False, sim_require_nnan=False, nc=nc)
        return tuple(outs)

    devices = jax.devices()[:N_CORES]
    mesh = Mesh(np.asarray(devices), ("core",))
    nin = n_params + len(out_names)
    fn = jax.jit(shard_map(_body, mesh=mesh,
                           in_specs=(PartitionSpec("core"),) * nin,
                           out_specs=(PartitionSpec("core"),) * len(out_names),
                           check_rep=False),
                 keep_unused=True)
    _STATE.update(fn=fn, in_names=in_names, out_names=out_names,
                  out_avals=out_avals, mesh=mesh,
                  sharding=NamedSharding(mesh, PartitionSpec("core")))
    return _STATE


def _run(in_maps):
    st = _get_exec()
    concat_in = [np.concatenate([np.asarray(in_maps[c][n]).reshape(-1)
                                 if np.asarray(in_maps[c][n]).ndim == 1
                                 else np.asarray(in_maps[c][n])
                                 for c in range(N_CORES)], axis=0)
                 for n in st["in_names"]]
    zeros = [np.zeros((N_CORES * a.shape[0], *a.shape[1:]), a.dtype)
             for a in st["out_avals"]]
    outs = st["fn"](*concat_in, *zeros)
    res = []
    for c in range(N_CORES):
        d = {}
        for i, n in enumerate(st["out_names"]):
            a = st["out_avals"][i]
            d[n] = np.asarray(outs[i]).reshape(N_CORES, *a.shape)[c]
        res.append(d)
    return res


def assemble(results):
    outs = np.zeros((3, TD), np.float32)
    for c in range(N_CORES):
        A = A_STEP * c
        n = min(L, TD - A)
        o = results[c]["out"]
        for u in range(3):
            outs[u, A:A + n] = o[u][:n]
    return outs.reshape(3, T, D)


def kernel(feature, W_w, W_b, Wx_w, Wx_b):
    in_maps = make_in_maps(feature, W_w, W_b, Wx_w, Wx_b)
    res = _run(in_maps)
    return assemble(res)


def hw_exec_time_ns(in_maps=None, iters=24):
    """Amortized per-execution device time: inputs resident on device,
    `iters` executions dispatched back-to-back (pipelined), minus the
    single-dispatch overhead baseline."""
    import time, jax
    st = _get_exec()
    if in_maps is None:
        rng = np.random.default_rng(0)
        feature = rng.standard_normal((3, T, D), dtype=np.float32)
        s = 1.0 / np.sqrt(6)
        in_maps = make_in_maps(feature,
                               rng.uniform(-s, s, (3, 6)).astype(np.float32),
                               rng.uniform(-s, s, 3).astype(np.float32),
                               rng.uniform(-s, s, (3, 6)).astype(np.float32),
                               rng.uniform(-s, s, 3).astype(np.float32))
    concat_in = [np.concatenate([np.asarray(in_maps[c][n]) for c in range(N_CORES)],
                                axis=0) for n in st["in_names"]]
    zeros = [np.zeros((N_CORES * a.shape[0], *a.shape[1:]), a.dtype)
             for a in st["out_avals"]]
    xd = [jax.device_put(a, st["sharding"]) for a in concat_in]
    zd = [jax.device_put(a, st["sharding"]) for a in zeros]
    for a in xd:
        a.block_until_ready()
    o = st["fn"](*xd, *zd)
    o[0].block_until_ready()
    t0 = time.time()
    o = st["fn"](*xd, *zd)
    o[0].block_until_ready()
    t1 = time.time()
    single = t1 - t0
    t0 = time.time()
    outs = [st["fn"](*xd, *zd) for _ in range(iters)]
    outs[-1][0].block_until_ready()
    t1 = time.time()
    per = (t1 - t0 - single) / (iters - 1)
    return max(per, 0.0) * 1e9, single * 1e9
